# revision 9
# baseline (speedup 1.0000x reference)
"""Cross-layer transcoder kernel for Trainium2 (8 NeuronCores, SPMD).

Math (from the reference):
    feats[l] = relu(x[l] @ W_enc[l].T + b_enc[l])          # [B, F] per layer
    recon[j] = sum_{i<=j} feats[i] @ W_dec[i, j] + b_dec[j] # [B, D] per layer

Sharding: the transcoder feature dim F=4096 is split across the 8 cores
(512 features each). Each core encodes its feature slice for all layers and
computes a partial reconstruction for every destination layer; the partials
are summed on the host (the gather/unshard step), where b_dec and the fp8
centering correction are also added.

Precision: encode and most of the decode run bf16 with fp32 PSUM
accumulation. Eleven decode pairs (i, j) run as fp8e4 DoubleRow matmuls
(K=256 per instruction, 2x PE throughput). For those pairs the feats are
centered and scaled (g = (f - 0.25) * 32, with 0.25*32 = 8.0 exactly
representable in e4m3 so the ~50% exact-zero feats stay exact) and
W_dec is scaled by 1024 before e4m3 quantization; the scaled psum partial
is rescaled by 2^-15 and added to the bf16 partial on-device. The exact
centering term 0.25 * colsum(q8(W_dec)) is added on the host. Measured
end-to-end relative error vs the fp32 reference is ~1.85e-2 (gate 2e-2).

Scheduling notes:
  - j<=6 output DMAs are issued by the gpsimd sequencer (SWDGE): its ~9us
    end-of-program drain hides under the 78us of decode j=7. j=7 outputs
    go via sync, which has no input loads left by then and drains fast.
    Issuing outputs from the scalar queue stalled encode RELUs behind
    saturated DMA rings.
  - DoubleRow matmuls run first in each psum group so the descale copy
    overlaps the bf16 matmuls instead of extending the tail.
  - b_enc is packed into one [128, 32] tile (one descriptor per partition
    row) instead of 64 [128, 1] tiles (128 4-byte descriptors each).
  - x tiles are split into 512-position halves so the first encode psum
    groups start earlier; the 28-matmul warmup covers the ~16.5us until
    layer 0's tiles land while ramping the PE clock to 2.4 GHz.
  - Output partials are written bf16 and summed in fp32 on the host.
"""

import os

import numpy as np
import ml_dtypes

L = 8          # n_layers
B = 1024       # n_pos
D = 768        # d_model
F = 4096       # d_transcoder
NCORES = 8
FL = F // NCORES   # features per core = 512
P = 128
KD = D // P        # 6  encode contraction chunks
MF = FL // P       # 4  feature chunks per core
MD = D // P        # 6  decode output chunks
NB = B // 512      # 2  position chunks of 512

# Decode pairs computed in fp8 DoubleRow; the rest run bf16.
FP8_PAIRS = [(0, j) for j in range(L)] + [(1, 5), (1, 6), (1, 7)]
FP8_SET = set(FP8_PAIRS)
FP8_LAYERS = sorted({i for i, _ in FP8_PAIRS})      # layers needing fp8 feats
BF_PAIRS = [
    (i, j) for j in range(L) for i in range(j + 1) if (i, j) not in FP8_SET
]
NBF = len(BF_PAIRS)     # 25
BF_IDX = {p: k for k, p in enumerate(BF_PAIRS)}

SF = 32.0               # feats fp8 scale
SW = 1024.0             # W_dec fp8 scale
CEN = 0.25              # feats centering offset (CEN*SF = 8.0, exact in e4m3)
DESCALE = 1.0 / (SF * SW)

BF16 = ml_dtypes.bfloat16
F8 = ml_dtypes.float8_e4m3

# Filled by the first kernel() call; reused afterwards.
_PROGRAM = None
# Stash of the most recent run's profiling results (test.py reads these).
LAST_EXEC_NS = None
LAST_RESULTS = None


def _build_program():
    import concourse.bacc as bacc
    import concourse.mybir as mybir
    import concourse.tile as tile

    nc = bacc.Bacc("TRN2", target_bir_lowering=False, debug=False)
    bf = mybir.dt.bfloat16
    f8 = mybir.dt.float8e4
    f32 = mybir.dt.float32

    xT_d = nc.dram_tensor("xT", [L, KD, NB, P, 512], bf, kind="ExternalInput")
    wencT_d = nc.dram_tensor("wencT", [L, KD, P, FL], bf, kind="ExternalInput")
    benc_d = nc.dram_tensor("benc", [P, L * MF], f32, kind="ExternalInput")
    wdec_d = nc.dram_tensor("wdec", [NBF, MF, P, D], bf, kind="ExternalInput")
    wdec8_d = nc.dram_tensor(
        "wdec8", [len(FP8_PAIRS), 2, P, 2, D], f8, kind="ExternalInput"
    )
    out_d = nc.dram_tensor("outT", [L, D, B], bf, kind="ExternalOutput")

    relu = mybir.ActivationFunctionType.Relu
    copyf = mybir.ActivationFunctionType.Copy
    addop = mybir.AluOpType.add
    mulop = mybir.AluOpType.mult
    subop = mybir.AluOpType.subtract
    dr = mybir.MatmulPerfMode.DoubleRow

    with tile.TileContext(nc) as tc:
        with (
            tc.tile_pool(name="feats", bufs=1) as feats_pool,
            tc.tile_pool(name="benc", bufs=1) as benc_pool,
            tc.tile_pool(name="xt", bufs=36) as xt_pool,
            tc.tile_pool(name="wenc", bufs=18) as wenc_pool,
            tc.tile_pool(name="wdec", bufs=32) as wdec_pool,
            tc.tile_pool(name="wdec8", bufs=8) as wdec8_pool,
            tc.tile_pool(name="outs", bufs=8) as out_pool,
            tc.tile_pool(name="tmp", bufs=4) as tmp_pool,
            tc.tile_pool(name="psum", bufs=8, space="PSUM") as psum_pool,
        ):
            # Bias tile: one DMA for all layers' encode biases.
            bt = benc_pool.tile([P, L * MF], f32, name="benc")
            nc.sync.dma_start(bt, benc_d[:])

            # Warm up the tensor engine during the prologue DMA fill: layer
            # 0's tiles land ~16.5us in, and HAM otherwise holds the PE at
            # 1.2 GHz for its first ~4.5us of work. These dummy matmuls
            # depend only on a memset tile, so they run from t~8us.
            warm = feats_pool.tile([P, 512], bf, name="warm")
            nc.vector.memset(warm, 0)
            wps = psum_pool.tile([P, 512], f32, name="wps", tag="psum")
            for w in range(30):
                nc.tensor.matmul(
                    wps,
                    lhsT=warm[:, :P],
                    rhs=warm,
                    start=(w == 0),
                    stop=(w == 29),
                )

            feats = {}
            ft8 = {i: {} for i in FP8_LAYERS}
            for j in range(L):
                # ---------- encode layer j into feats[(j, mf)] ----------
                xts = {}
                wes = []
                for kd in range(KD):
                    xt = xt_pool.tile([P, 512], bf, name="xt", tag="xt")
                    nc.sync.dma_start(xt, xT_d[j, kd, 0])
                    xts[(kd, 0)] = xt
                    we = wenc_pool.tile([P, FL], bf, name="we", tag="we")
                    nc.sync.dma_start(we, wencT_d[j, kd])
                    wes.append(we)
                for kd in range(KD):
                    xt = xt_pool.tile([P, 512], bf, name="xt", tag="xt")
                    nc.sync.dma_start(xt, xT_d[j, kd, 1])
                    xts[(kd, 1)] = xt
                for mf in range(MF):
                    ft = feats_pool.tile([P, B], bf, name=f"feat_{j}_{mf}")
                    feats[(j, mf)] = ft
                if j in FP8_LAYERS:
                    # Centered fp8 copy of this layer's feats for DoubleRow:
                    # ft8[j][c][p, s, b] = (feats[f=c*256+s*128+p, b]-CEN)*SF
                    for c in range(2):
                        ft8[j][c] = feats_pool.tile(
                            [P, 2, B], f8, name=f"ft8_{j}_{c}"
                        )
                for nb in range(NB):
                    for mf in range(MF):
                        ps = psum_pool.tile([P, 512], f32, name="ps", tag="psum")
                        for kd in range(KD):
                            nc.tensor.matmul(
                                ps,
                                lhsT=wes[kd][:, mf * P:(mf + 1) * P],
                                rhs=xts[(kd, nb)],
                                start=(kd == 0),
                                stop=(kd == KD - 1),
                            )
                        idx = j * MF + mf
                        bsl = slice(nb * 512, (nb + 1) * 512)
                        nc.scalar.activation(
                            feats[(j, mf)][:, bsl],
                            ps,
                            relu,
                            bias=bt[:, idx:idx + 1],
                        )
                        if j in FP8_LAYERS:
                            nc.vector.tensor_scalar(
                                ft8[j][mf >> 1][:, mf & 1, bsl],
                                feats[(j, mf)][:, bsl],
                                SF,
                                CEN * SF,
                                mulop,
                                subop,
                            )

                # ---------- decode destination layer j ----------
                fp8_is = [i for i in FP8_LAYERS if (i, j) in FP8_SET]
                bf_is = [i for i in range(j + 1) if (i, j) not in FP8_SET]
                nmm = len(bf_is) * MF
                n8 = len(fp8_is) * 2
                wt8 = {}
                for i in fp8_is:
                    p8 = FP8_PAIRS.index((i, j))
                    for c in range(2):
                        w8 = wdec8_pool.tile([P, 2, D], f8, name="wd8", tag="wd8")
                        nc.sync.dma_start(w8, wdec8_d[p8, c])
                        wt8[(i, c)] = w8
                wts = {}
                for i in bf_is:
                    pidx = BF_IDX[(i, j)]
                    for kf in range(MF):
                        wt = wdec_pool.tile([P, D], bf, name="wd", tag="wd")
                        nc.sync.dma_start(wt, wdec_d[pidx, kf])
                        wts[(i, kf)] = wt
                for nb in range(NB):
                    for md in range(MD):
                        dsl = slice(md * P, (md + 1) * P)
                        bsl = slice(nb * 512, (nb + 1) * 512)
                        if n8:
                            ps8 = psum_pool.tile(
                                [P, 512], f32, name="ps8", tag="psum"
                            )
                            cnt8 = 0
                            for i in fp8_is:
                                for c in range(2):
                                    nc.tensor.matmul(
                                        ps8,
                                        lhsT=wt8[(i, c)][:, :, dsl],
                                        rhs=ft8[i][c][:, :, bsl],
                                        start=(cnt8 == 0),
                                        stop=(cnt8 == n8 - 1),
                                        perf_mode=dr,
                                    )
                                    cnt8 += 1
                        if nmm:
                            ps = psum_pool.tile(
                                [P, 512], f32, name="ps", tag="psum"
                            )
                            cnt = 0
                            for i in bf_is:
                                for kf in range(MF):
                                    nc.tensor.matmul(
                                        ps,
                                        lhsT=wts[(i, kf)][:, dsl],
                                        rhs=feats[(i, kf)][:, bsl],
                                        start=(cnt == 0),
                                        stop=(cnt == nmm - 1),
                                    )
                                    cnt += 1
                        ot = out_pool.tile([P, 512], bf, name="ot", tag="ot")
                        final = j == L - 1 and nb == NB - 1 and md == MD - 1
                        if final:
                            # tail: store each half as soon as its add lands
                            tmp = tmp_pool.tile([P, 512], bf, name="tm", tag="tm")
                            nc.scalar.activation(tmp, ps8, copyf, scale=DESCALE)
                            for h in range(2):
                                hs = slice(h * 256, (h + 1) * 256)
                                nc.vector.tensor_tensor(
                                    ot[:, hs], ps[:, hs], tmp[:, hs], addop
                                )
                                nc.sync.dma_start(
                                    out_d[j, dsl, nb * 512 + h * 256:
                                          nb * 512 + (h + 1) * 256],
                                    ot[:, hs],
                                )
                            continue
                        if n8 and nmm:
                            tmp = tmp_pool.tile([P, 512], bf, name="tm", tag="tm")
                            nc.scalar.activation(tmp, ps8, copyf, scale=DESCALE)
                            nc.vector.tensor_tensor(ot, ps, tmp, addop)
                        elif n8:
                            nc.vector.tensor_scalar_mul(ot, ps8, DESCALE)
                        else:
                            nc.vector.tensor_copy(ot, ps)
                        # j<=6 partials drain via the gpsimd SWDGE queue (its
                        # ~9us end-of-program drain hides under decode j=7);
                        # j=7 partials go via sync, which has no input loads
                        # left to issue by then and drains fast.
                        if j < L - 1:
                            nc.gpsimd.dma_start(out_d[j, dsl, bsl], ot)
                        else:
                            nc.sync.dma_start(out_d[j, dsl, bsl], ot)

    nc.compile()
    return nc


def _prepare_inputs(x, W_enc, b_enc, W_dec):
    """Host-side shard + pack + cast. Returns in_maps for the 8 cores."""
    xT = np.ascontiguousarray(
        x.transpose(0, 2, 1)
        .reshape(L, KD, P, NB, 512)
        .transpose(0, 1, 3, 2, 4)
    ).astype(BF16)
    in_maps = []
    for c in range(NCORES):
        s = slice(c * FL, (c + 1) * FL)
        wencT = (
            np.ascontiguousarray(W_enc[:, s, :].transpose(0, 2, 1))
            .astype(BF16)
            .reshape(L, KD, P, FL)
        )
        benc = np.ascontiguousarray(
            b_enc[:, s].reshape(L, MF, P).transpose(2, 0, 1).reshape(P, L * MF),
            dtype=np.float32,
        )
        wdec = np.empty((NBF, MF, P, D), dtype=BF16)
        for k, (i, j) in enumerate(BF_PAIRS):
            wdec[k] = W_dec[i, j, s, :].astype(BF16).reshape(MF, P, D)
        wdec8 = np.empty((len(FP8_PAIRS), 2, P, 2, D), dtype=F8)
        for k, (i, j) in enumerate(FP8_PAIRS):
            w = np.clip(W_dec[i, j, s, :] * SW, -240, 240)
            wdec8[k] = w.reshape(2, 2, P, D).transpose(0, 2, 1, 3).astype(F8)
        in_maps.append(
            {
                "xT": xT,
                "wencT": wencT,
                "benc": benc,
                "wdec": wdec,
                "wdec8": wdec8,
            }
        )
    return in_maps


def kernel(x, W_enc, b_enc, W_dec, b_dec):
    global _PROGRAM, LAST_EXEC_NS, LAST_RESULTS
    from concourse import bass_utils

    x = np.asarray(x)
    W_enc = np.asarray(W_enc)
    b_enc = np.asarray(b_enc)
    W_dec = np.asarray(W_dec)
    b_dec = np.asarray(b_dec)

    if _PROGRAM is None:
        _PROGRAM = _build_program()
    nc = _PROGRAM

    in_maps = _prepare_inputs(x, W_enc, b_enc, W_dec)

    # Exact centering correction: each fp8 pair computed (f - CEN) @ q8(W),
    # so add CEN * colsum(q8(W)) (summed over all cores' feature slices).
    corr = np.zeros((L, D), dtype=np.float32)
    for m in in_maps:
        w8 = m["wdec8"].astype(np.float32)            # [npair, 2, P, 2, D]
        colsum = w8.sum(axis=(1, 2, 3)) * (CEN / SW)  # [npair, D]
        for k, (i, j) in enumerate(FP8_PAIRS):
            corr[j] += colsum[k]

    trace = os.environ.get("KERNEL_TRACE", "0") == "1"
    res = bass_utils.run_bass_kernel_spmd(
        nc, in_maps, core_ids=list(range(NCORES)), trace=trace
    )
    LAST_EXEC_NS = res.exec_time_ns
    LAST_RESULTS = res

    acc = np.zeros((L, D, B), dtype=np.float32)
    for r in res.results:
        acc += np.asarray(r["outT"], dtype=np.float32)
    out = (
        acc.transpose(0, 2, 1)
        + b_dec.astype(np.float32)[:, None, :]
        + corr[:, None, :]
    )
    return np.ascontiguousarray(out, dtype=np.float32)


# revision 10
# speedup vs baseline: 1.1971x; 1.1971x over previous
"""Cross-layer transcoder kernel for Trainium2 (8 NeuronCores, SPMD).

Math (from the reference):
    feats[l] = relu(x[l] @ W_enc[l].T + b_enc[l])          # [B, F] per layer
    recon[j] = sum_{i<=j} feats[i] @ W_dec[i, j] + b_dec[j] # [B, D] per layer

Sharding: the transcoder feature dim F=4096 is split across the 8 cores
(512 features each). Each core encodes its feature slice for all layers and
computes a partial reconstruction for every destination layer; the partials
are summed on the host (the gather/unshard step), where b_dec and the fp8
centering correction are also added.

Precision: encode and most of the decode run bf16 with fp32 PSUM
accumulation. Eleven decode pairs (i, j) run as fp8e4 DoubleRow matmuls
(K=256 per instruction, 2x PE throughput). For those pairs the feats are
centered and scaled (g = (f - 0.25) * 32, with 0.25*32 = 8.0 exactly
representable in e4m3 so the ~50% exact-zero feats stay exact) and
W_dec is scaled by 1024 before e4m3 quantization; the scaled psum partial
is rescaled by 2^-15 and added to the bf16 partial on-device. The exact
centering term 0.25 * colsum(q8(W_dec)) is added on the host. Measured
end-to-end relative error vs the fp32 reference is ~1.85e-2 (gate 2e-2).

Scheduling notes:
  - j<=6 output DMAs are issued by the gpsimd sequencer (SWDGE): its ~9us
    end-of-program drain hides under the 78us of decode j=7. j=7 outputs
    go via sync, which has no input loads left by then and drains fast.
    Issuing outputs from the scalar queue stalled encode RELUs behind
    saturated DMA rings.
  - DoubleRow matmuls run first in each psum group so the descale copy
    overlaps the bf16 matmuls instead of extending the tail.
  - b_enc is packed into one [128, 32] tile (one descriptor per partition
    row) instead of 64 [128, 1] tiles (128 4-byte descriptors each).
  - x tiles are split into 512-position halves so the first encode psum
    groups start earlier; the 28-matmul warmup covers the ~16.5us until
    layer 0's tiles land while ramping the PE clock to 2.4 GHz.
  - Output partials are written bf16 and summed in fp32 on the host.
"""

import os

import numpy as np
import ml_dtypes

L = 8          # n_layers
B = 1024       # n_pos
D = 768        # d_model
F = 4096       # d_transcoder
NCORES = 8
FL = F // NCORES   # features per core = 512
P = 128
KD = D // P        # 6  encode contraction chunks
MF = FL // P       # 4  feature chunks per core
MD = D // P        # 6  decode output chunks
NB = B // 512      # 2  position chunks of 512

# Decode pairs computed in fp8 DoubleRow; the rest run bf16.
FP8_PAIRS = [(0, j) for j in range(L)] + [(1, 5), (1, 6), (1, 7)]
FP8_SET = set(FP8_PAIRS)
FP8_LAYERS = sorted({i for i, _ in FP8_PAIRS})      # layers needing fp8 feats
BF_PAIRS = [
    (i, j) for j in range(L) for i in range(j + 1) if (i, j) not in FP8_SET
]
NBF = len(BF_PAIRS)     # 25
BF_IDX = {p: k for k, p in enumerate(BF_PAIRS)}

SF = 32.0               # feats fp8 scale
SW = 1024.0             # W_dec fp8 scale
CEN = 0.25              # feats centering offset (CEN*SF = 8.0, exact in e4m3)
DESCALE = 1.0 / (SF * SW)

BF16 = ml_dtypes.bfloat16
F8 = ml_dtypes.float8_e4m3

# Filled by the first kernel() call; reused afterwards.
_PROGRAM = None
# Stash of the most recent run's profiling results (test.py reads these).
LAST_EXEC_NS = None
LAST_RESULTS = None


def _build_program():
    import concourse.bacc as bacc
    import concourse.mybir as mybir
    import concourse.tile as tile

    nc = bacc.Bacc("TRN2", target_bir_lowering=False, debug=False)
    bf = mybir.dt.bfloat16
    f8 = mybir.dt.float8e4
    f32 = mybir.dt.float32

    xT_d = nc.dram_tensor("xT", [L, KD, NB, P, 512], bf, kind="ExternalInput")
    wencT_d = nc.dram_tensor("wencT", [L, KD, P, FL], bf, kind="ExternalInput")
    benc_d = nc.dram_tensor("benc", [P, L * MF], f32, kind="ExternalInput")
    wdec_d = nc.dram_tensor("wdec", [NBF, MF, P, D], bf, kind="ExternalInput")
    wdec8_d = nc.dram_tensor(
        "wdec8", [len(FP8_PAIRS), 2, P, 2, D], f8, kind="ExternalInput"
    )
    out_d = nc.dram_tensor("outT", [L, D, B], bf, kind="ExternalOutput")

    relu = mybir.ActivationFunctionType.Relu
    copyf = mybir.ActivationFunctionType.Copy
    addop = mybir.AluOpType.add
    mulop = mybir.AluOpType.mult
    subop = mybir.AluOpType.subtract
    dr = mybir.MatmulPerfMode.DoubleRow

    with tile.TileContext(nc) as tc:
        with (
            tc.tile_pool(name="feats", bufs=1) as feats_pool,
            tc.tile_pool(name="benc", bufs=1) as benc_pool,
            tc.tile_pool(name="xt", bufs=36) as xt_pool,
            tc.tile_pool(name="wenc", bufs=18) as wenc_pool,
            tc.tile_pool(name="wdec", bufs=32) as wdec_pool,
            tc.tile_pool(name="wdec8", bufs=8) as wdec8_pool,
            tc.tile_pool(name="outs", bufs=8) as out_pool,
            tc.tile_pool(name="tmp", bufs=4) as tmp_pool,
            tc.tile_pool(name="psum", bufs=8, space="PSUM") as psum_pool,
        ):
            # Bias tile: one DMA for all layers' encode biases.
            bt = benc_pool.tile([P, L * MF], f32, name="benc")
            nc.sync.dma_start(bt, benc_d[:])

            # Warm up the tensor engine during the prologue DMA fill: layer
            # 0's tiles land ~16.5us in, and HAM otherwise holds the PE at
            # 1.2 GHz for its first ~4.5us of work. These dummy matmuls
            # depend only on a memset tile, so they run from t~8us.
            warm = feats_pool.tile([P, 512], bf, name="warm")
            nc.vector.memset(warm, 0)
            wps = psum_pool.tile([P, 512], f32, name="wps", tag="psum")
            for w in range(28):
                nc.tensor.matmul(
                    wps,
                    lhsT=warm[:, :P],
                    rhs=warm,
                    start=(w == 0),
                    stop=(w == 27),
                )

            feats = {}
            ft8 = {i: {} for i in FP8_LAYERS}
            for j in range(L):
                # ---------- encode layer j into feats[(j, mf)] ----------
                xts = {}
                wes = []
                for kd in range(KD):
                    xt = xt_pool.tile([P, 512], bf, name="xt", tag="xt")
                    nc.sync.dma_start(xt, xT_d[j, kd, 0])
                    xts[(kd, 0)] = xt
                    we = wenc_pool.tile([P, FL], bf, name="we", tag="we")
                    nc.sync.dma_start(we, wencT_d[j, kd])
                    wes.append(we)
                for kd in range(KD):
                    xt = xt_pool.tile([P, 512], bf, name="xt", tag="xt")
                    nc.sync.dma_start(xt, xT_d[j, kd, 1])
                    xts[(kd, 1)] = xt
                for mf in range(MF):
                    ft = feats_pool.tile([P, B], bf, name=f"feat_{j}_{mf}")
                    feats[(j, mf)] = ft
                if j in FP8_LAYERS:
                    # Centered fp8 copy of this layer's feats for DoubleRow:
                    # ft8[j][c][p, s, b] = (feats[f=c*256+s*128+p, b]-CEN)*SF
                    for c in range(2):
                        ft8[j][c] = feats_pool.tile(
                            [P, 2, B], f8, name=f"ft8_{j}_{c}"
                        )
                for nb in range(NB):
                    for mf in range(MF):
                        ps = psum_pool.tile([P, 512], f32, name="ps", tag="psum")
                        for kd in range(KD):
                            nc.tensor.matmul(
                                ps,
                                lhsT=wes[kd][:, mf * P:(mf + 1) * P],
                                rhs=xts[(kd, nb)],
                                start=(kd == 0),
                                stop=(kd == KD - 1),
                            )
                        idx = j * MF + mf
                        bsl = slice(nb * 512, (nb + 1) * 512)
                        nc.scalar.activation(
                            feats[(j, mf)][:, bsl],
                            ps,
                            relu,
                            bias=bt[:, idx:idx + 1],
                        )
                        if j in FP8_LAYERS:
                            nc.vector.tensor_scalar(
                                ft8[j][mf >> 1][:, mf & 1, bsl],
                                feats[(j, mf)][:, bsl],
                                SF,
                                CEN * SF,
                                mulop,
                                subop,
                            )

                # ---------- decode destination layer j ----------
                fp8_is = [i for i in FP8_LAYERS if (i, j) in FP8_SET]
                bf_is = [i for i in range(j + 1) if (i, j) not in FP8_SET]
                nmm = len(bf_is) * MF
                n8 = len(fp8_is) * 2
                wt8 = {}
                for i in fp8_is:
                    p8 = FP8_PAIRS.index((i, j))
                    for c in range(2):
                        w8 = wdec8_pool.tile([P, 2, D], f8, name="wd8", tag="wd8")
                        nc.sync.dma_start(w8, wdec8_d[p8, c])
                        wt8[(i, c)] = w8
                wts = {}
                for i in bf_is:
                    pidx = BF_IDX[(i, j)]
                    for kf in range(MF):
                        wt = wdec_pool.tile([P, D], bf, name="wd", tag="wd")
                        nc.sync.dma_start(wt, wdec_d[pidx, kf])
                        wts[(i, kf)] = wt
                for nb in range(NB):
                    for md in range(MD):
                        dsl = slice(md * P, (md + 1) * P)
                        bsl = slice(nb * 512, (nb + 1) * 512)
                        if n8:
                            ps8 = psum_pool.tile(
                                [P, 512], f32, name="ps8", tag="psum"
                            )
                            cnt8 = 0
                            for i in fp8_is:
                                for c in range(2):
                                    nc.tensor.matmul(
                                        ps8,
                                        lhsT=wt8[(i, c)][:, :, dsl],
                                        rhs=ft8[i][c][:, :, bsl],
                                        start=(cnt8 == 0),
                                        stop=(cnt8 == n8 - 1),
                                        perf_mode=dr,
                                    )
                                    cnt8 += 1
                        if nmm:
                            ps = psum_pool.tile(
                                [P, 512], f32, name="ps", tag="psum"
                            )
                            cnt = 0
                            for i in bf_is:
                                for kf in range(MF):
                                    nc.tensor.matmul(
                                        ps,
                                        lhsT=wts[(i, kf)][:, dsl],
                                        rhs=feats[(i, kf)][:, bsl],
                                        start=(cnt == 0),
                                        stop=(cnt == nmm - 1),
                                    )
                                    cnt += 1
                        ot = out_pool.tile([P, 512], bf, name="ot", tag="ot")
                        if n8 and nmm:
                            tmp = tmp_pool.tile([P, 512], bf, name="tm", tag="tm")
                            nc.scalar.activation(tmp, ps8, copyf, scale=DESCALE)
                            nc.vector.tensor_tensor(ot, ps, tmp, addop)
                        elif n8:
                            nc.vector.tensor_scalar_mul(ot, ps8, DESCALE)
                        else:
                            nc.vector.tensor_copy(ot, ps)
                        # j<=6 partials drain via the gpsimd SWDGE queue (its
                        # ~9us end-of-program drain hides under decode j=7);
                        # j=7 partials go via sync, which has no input loads
                        # left to issue by then and drains fast.
                        if j < L - 1:
                            nc.gpsimd.dma_start(out_d[j, dsl, bsl], ot)
                        else:
                            nc.sync.dma_start(out_d[j, dsl, bsl], ot)

    nc.compile()
    return nc


def _prepare_inputs(x, W_enc, b_enc, W_dec):
    """Host-side shard + pack + cast. Returns in_maps for the 8 cores."""
    xT = np.ascontiguousarray(
        x.transpose(0, 2, 1)
        .reshape(L, KD, P, NB, 512)
        .transpose(0, 1, 3, 2, 4)
    ).astype(BF16)
    in_maps = []
    for c in range(NCORES):
        s = slice(c * FL, (c + 1) * FL)
        wencT = (
            np.ascontiguousarray(W_enc[:, s, :].transpose(0, 2, 1))
            .astype(BF16)
            .reshape(L, KD, P, FL)
        )
        benc = np.ascontiguousarray(
            b_enc[:, s].reshape(L, MF, P).transpose(2, 0, 1).reshape(P, L * MF),
            dtype=np.float32,
        )
        wdec = np.empty((NBF, MF, P, D), dtype=BF16)
        for k, (i, j) in enumerate(BF_PAIRS):
            wdec[k] = W_dec[i, j, s, :].astype(BF16).reshape(MF, P, D)
        wdec8 = np.empty((len(FP8_PAIRS), 2, P, 2, D), dtype=F8)
        for k, (i, j) in enumerate(FP8_PAIRS):
            w = np.clip(W_dec[i, j, s, :] * SW, -240, 240)
            wdec8[k] = w.reshape(2, 2, P, D).transpose(0, 2, 1, 3).astype(F8)
        in_maps.append(
            {
                "xT": xT,
                "wencT": wencT,
                "benc": benc,
                "wdec": wdec,
                "wdec8": wdec8,
            }
        )
    return in_maps


def kernel(x, W_enc, b_enc, W_dec, b_dec):
    global _PROGRAM, LAST_EXEC_NS, LAST_RESULTS
    from concourse import bass_utils

    x = np.asarray(x)
    W_enc = np.asarray(W_enc)
    b_enc = np.asarray(b_enc)
    W_dec = np.asarray(W_dec)
    b_dec = np.asarray(b_dec)

    if _PROGRAM is None:
        _PROGRAM = _build_program()
    nc = _PROGRAM

    in_maps = _prepare_inputs(x, W_enc, b_enc, W_dec)

    # Exact centering correction: each fp8 pair computed (f - CEN) @ q8(W),
    # so add CEN * colsum(q8(W)) (summed over all cores' feature slices).
    corr = np.zeros((L, D), dtype=np.float32)
    for m in in_maps:
        w8 = m["wdec8"].astype(np.float32)            # [npair, 2, P, 2, D]
        colsum = w8.sum(axis=(1, 2, 3)) * (CEN / SW)  # [npair, D]
        for k, (i, j) in enumerate(FP8_PAIRS):
            corr[j] += colsum[k]

    trace = os.environ.get("KERNEL_TRACE", "0") == "1"
    res = bass_utils.run_bass_kernel_spmd(
        nc, in_maps, core_ids=list(range(NCORES)), trace=trace
    )
    LAST_EXEC_NS = res.exec_time_ns
    LAST_RESULTS = res

    acc = np.zeros((L, D, B), dtype=np.float32)
    for r in res.results:
        acc += np.asarray(r["outT"], dtype=np.float32)
    out = (
        acc.transpose(0, 2, 1)
        + b_dec.astype(np.float32)[:, None, :]
        + corr[:, None, :]
    )
    return np.ascontiguousarray(out, dtype=np.float32)
